# revision 13
# baseline (speedup 1.0000x reference)
"""Trainium2 Bass kernel for the DCHBlock (dilated conv + multi-head dilated
state + GLU FFN residual block).

Sharding: pure data parallel over batch — 8 samples, 8 NeuronCores, one
sample per core, weights replicated. No collectives.

Layout: channel-major trunk [384ch -> 3 x [128, 2048]] in SBUF. All convs and
dense matmuls run on the PE in float32r (TF32-like, 1 cyc/row at N=512).
Dilated causal convs are 4 time-shifted PSUM-accumulated matmuls with
partial-window clamping at the left edge. LayerNorm stats use ones-vector
PE matmuls (cross-partition reduce) + GPSIMD partition_broadcast; the
rsqrt runs on DVE (bit-trick + Newton) so the scalar engine stays on one
activation-table set (gelu/tanh/square/copy) the whole kernel. Sigmoid is
computed as 0.5*tanh(x/2)+0.5 with the affine folded into downstream
weights. The paired dilated-state heads run concurrently on the PE via
disjoint 64x64 array quadrants (rows 0:48 / 64:112)."""

from contextlib import ExitStack

import ml_dtypes
import numpy as np

import concourse.bass as bass
import concourse.mybir as mybir
import concourse.tile as tile
from concourse import bacc
from concourse.bass_utils import run_bass_kernel_spmd

F32 = mybir.dt.float32
I32 = mybir.dt.int32
F32R = mybir.dt.float32r
BF16 = mybir.dt.bfloat16
AF = mybir.ActivationFunctionType
ALU = mybir.AluOpType
ts = bass.ts

S = 2048
H = 384
K = 4
NH = 8
HD = 48
INNER = 4 * H            # 1536
NCH = H // 128           # 3 channel chunks
NTT = S // 512           # 4 time tiles of 512
NPC = INNER // 128       # 12
STACK_DIL = (1, 2, 4, 8, 16, 32)
HEAD_DIL = ((1, 2, 4), (4, 8, 16), (16, 32, 64), (64, 128, 256),
            (256, 512, 1024), (1, 16, 256), (4, 64, 1024), (16, 256, 2048))
EPS = 1e-5
QMAGIC = 0x5F3759DF

# consts tile column map: [128, NCONST] f32
#  cols 0..17   : conv stack bias  (l, oc) -> 3*l + oc          (rows = out ch)
#  cols 18..29  : head bias (x2)   (sc, j) -> 18 + 3*sc + j     (rows 0:48/64:112)
#  cols 30..32  : mix bias         oc      -> 30 + oc
#  cols 33..35  : proj bias        oc      -> 33 + oc
NCONST = 36


def _win(tt, s):
    """Valid source window for output tile tt (cols [tt*512, tt*512+512))
    of a causal tap with left shift s. Returns (src_start, n, dst_off) or
    None if fully out of range."""
    lo = tt * 512 - s
    if lo + 512 <= 0:
        return None
    if lo >= 0:
        return (lo, 512, 0)
    return (0, 512 + lo, -lo)


def _build(apply_lnga=(False, False, False), dbg=False, reps=1):
    nc = bacc.Bacc(None, target_bir_lowering=False)

    t = {}
    t["X"] = nc.dram_tensor("X", [S, H], F32R, kind="ExternalInput")
    t["WST"] = nc.dram_tensor("WST", [6, 128, NCH, K, H], F32R, kind="ExternalInput")
    t["PROJT"] = nc.dram_tensor("PROJT", [NCH, 128, H], F32R, kind="ExternalInput")
    t["GWP"] = nc.dram_tensor("GWP", [NCH, 128, 1024], BF16, kind="ExternalInput")
    t["HWPA"] = nc.dram_tensor("HWPA", [4, 128, 3, K, 64], F32R, kind="ExternalInput")
    t["HWPB"] = nc.dram_tensor("HWPB", [4, 128, 3, K, 64], F32R, kind="ExternalInput")
    t["MIXW"] = nc.dram_tensor("MIXW", [4, 128, H], F32R, kind="ExternalInput")
    t["FIT"] = nc.dram_tensor("FIT", [NCH, 128, 2 * INNER], BF16, kind="ExternalInput")
    t["FOT"] = nc.dram_tensor("FOT", [NPC, 128, H], F32R, kind="ExternalInput")
    t["CONSTS"] = nc.dram_tensor("CONSTS", [128, NCONST], F32, kind="ExternalInput")
    t["LNG"] = nc.dram_tensor("LNG", [128, 9], F32, kind="ExternalInput")
    t["LNB"] = nc.dram_tensor("LNB", [128, 9], F32, kind="ExternalInput")
    t["ONESR"] = nc.dram_tensor("ONESR", [128, 1], F32R, kind="ExternalInput")
    t["IDENT"] = nc.dram_tensor("IDENT", [128, 128], F32R, kind="ExternalInput")
    t["ZPAD"] = nc.dram_tensor("ZPAD", [128, 96], F32R, kind="ExternalInput")
    t["OUT"] = nc.dram_tensor("OUT", [S, H], F32, kind="ExternalOutput")
    if dbg:
        for nm in ("DH", "DX1", "DST", "DX2"):
            shp = [4, 128, S] if nm == "DST" else [NCH, 128, S]
            t[nm] = nc.dram_tensor(nm, shp, F32, kind="ExternalOutput")

    with tile.TileContext(nc) as tc:
        _emit(nc, tc, t, apply_lnga, dbg, reps)
    nc.finalize()
    return nc


def _emit(nc, tc, t, apply_lnga, dbg, reps=1):
    ctx = ExitStack()
    with ctx:
        singles = ctx.enter_context(tc.tile_pool(name="singles", bufs=1))
        trunk = ctx.enter_context(tc.tile_pool(name="trunk", bufs=2))
        lnscr = ctx.enter_context(tc.tile_pool(name="lnscr", bufs=2))
        stat = ctx.enter_context(tc.tile_pool(name="stat", bufs=2))
        n2p = ctx.enter_context(tc.tile_pool(name="n2p", bufs=1))

        ident = singles.tile([128, 128], F32R)
        nc.sync.dma_start(out=ident[:], in_=t["IDENT"][:])
        consts = singles.tile([128, NCONST], F32)
        lng = singles.tile([128, 9], F32)
        lnb = singles.tile([128, 9], F32)
        ones = singles.tile([128, 1], F32R)

        def load_consts():
            nc.sync.dma_start(out=consts[:], in_=t["CONSTS"][:])
            nc.sync.dma_start(out=lng[:], in_=t["LNG"][:])
            nc.sync.dma_start(out=lnb[:], in_=t["LNB"][:])
            nc.sync.dma_start(out=ones[:], in_=t["ONESR"][:])

        def ln_norm_tt(pstat, xsrc, ln_idx, dst, tt, dst_off=0):
            """One time-tile of LN: stats via ones-matmul partition
            reduction; rsqrt on DVE (quake + 2 Newton); per-time
            broadcast on GPSIMD."""
            w = ts(tt, 512)
            ps = pstat.tile([1, 512], F32, tag="ps_s")
            pq = pstat.tile([1, 512], F32, tag="ps_q")
            for ic in range(NCH):
                sq = lnscr.tile([128, 512], F32R, tag="sq")
                nc.scalar.activation(out=sq[:], in_=xsrc[ic][:, w],
                                     func=AF.Square)
                nc.tensor.matmul(ps[:], ones[:], xsrc[ic][:, w],
                                 start=(ic == 0), stop=(ic == NCH - 1))
                nc.tensor.matmul(pq[:], ones[:], sq[:],
                                 start=(ic == 0), stop=(ic == NCH - 1))
            # m = ps/H on ACT (copies psum->sbuf; only one PSUM operand is
            # allowed per DVE tensor-tensor op), then var+eps on DVE
            sm = stat.tile([1, 512], F32, tag="sm")
            nc.scalar.activation(out=sm[:], in_=ps[:], func=AF.Copy,
                                 scale=1.0 / H)
            q1 = stat.tile([1, 512], F32, tag="sc1")
            nc.vector.tensor_mul(q1[:], sm[:], sm[:])
            w2 = stat.tile([1, 512], F32, tag="sc2")
            nc.vector.scalar_tensor_tensor(
                out=w2[:], in0=q1[:], scalar=-float(H), in1=pq[:],
                op0=ALU.mult, op1=ALU.add)
            vb = stat.tile([1, 512], F32, tag="vb")
            nc.vector.tensor_scalar(
                out=vb[:], in0=w2[:], scalar1=1.0 / H, scalar2=EPS,
                op0=ALU.mult, op1=ALU.add)
            # r = rsqrt(vb): quake initial + 2 Newton iterations
            yi = stat.tile([1, 512], I32, tag="yi")
            nc.vector.tensor_scalar(
                out=yi[:], in0=vb[:].bitcast(I32), scalar1=1, scalar2=None,
                op0=ALU.logical_shift_right)
            nc.vector.tensor_scalar(
                out=yi[:], in0=yi[:], scalar1=-1, scalar2=QMAGIC,
                op0=ALU.mult, op1=ALU.add)
            y0 = yi[:].bitcast(F32)
            r = None
            for it in range(2):
                ysq = stat.tile([1, 512], F32, tag="sc1")
                nc.vector.tensor_mul(ysq[:], y0, y0)
                hv = stat.tile([1, 512], F32, tag="sc2")
                nc.vector.scalar_tensor_tensor(
                    out=hv[:], in0=vb[:], scalar=-0.5, in1=ysq[:],
                    op0=ALU.mult, op1=ALU.mult)
                yn = stat.tile([1, 512], F32, tag="yn")
                nc.vector.scalar_tensor_tensor(
                    out=yn[:], in0=hv[:], scalar=1.5, in1=y0,
                    op0=ALU.add, op1=ALU.mult)
                y0 = yn[:]
                r = yn
            mr = stat.tile([1, 512], F32, tag="mr")
            nc.vector.tensor_mul(mr[:], sm[:], r[:])
            rb = lnscr.tile([128, 512], F32, tag="rb")
            nc.gpsimd.partition_broadcast(rb[:], r[:])
            mrb = lnscr.tile([128, 512], F32, tag="mrb")
            nc.gpsimd.partition_broadcast(mrb[:], mr[:])
            for ic in range(NCH):
                t1 = lnscr.tile([128, 512], F32, tag="t1")
                nc.vector.tensor_mul(t1[:], xsrc[ic][:, w], rb[:])
                dw = bass.ds(dst_off + tt * 512, 512)
                if apply_lnga[ln_idx]:
                    t2 = lnscr.tile([128, 512], F32, tag="t2")
                    nc.vector.tensor_sub(t2[:], t1[:], mrb[:])
                    gi = 3 * ln_idx + ic
                    nc.scalar.activation(
                        out=dst[ic][:, dw], in_=t2[:], func=AF.Identity,
                        bias=lnb[:, gi:gi + 1], scale=lng[:, gi:gi + 1])
                else:
                    nc.vector.tensor_sub(dst[ic][:, dw], t1[:], mrb[:])

        def dump_cm(tiles, dram, rows=128):
            for c, tl in enumerate(tiles):
                nc.sync.dma_start(out=dram[c, 0:rows, :].bitcast(tl.dtype),
                                  in_=tl[0:rows, :])

        def one_pass():
            load_consts()
            # ---- Phase 0: load + transpose x -> cm trunk x0 ----
            x0 = [trunk.tile([128, S], F32R, tag=f"x{c}", name=f"x{c}") for c in range(NCH)]
            with tc.tile_pool(name="p0", bufs=4) as p0, \
                 tc.tile_pool(name="ps0", bufs=4, space="PSUM") as ps0:
                for tch in range(S // 128):
                    xt = p0.tile([128, H], F32R, tag="xt")
                    nc.sync.dma_start(out=xt[:], in_=t["X"][ts(tch, 128), :])
                    for oc in range(NCH):
                        pt = ps0.tile([128, 128], F32R, tag="pt")
                        nc.tensor.transpose(pt[:], xt[:, ts(oc, 128)], ident[:])
                        nc.scalar.copy(out=x0[oc][:, ts(tch, 128)], in_=pt[:])

            # ---- Phase 1: conv stack branch ----
            with tc.tile_pool(name="hcv", bufs=2) as hcv, \
                 tc.tile_pool(name="wcv", bufs=2) as wcv, \
                 tc.tile_pool(name="gelp", bufs=2) as gelp, \
                 tc.tile_pool(name="prj", bufs=1) as prj, \
                 tc.tile_pool(name="pstat1", bufs=2, space="PSUM") as pstat1, \
                 tc.tile_pool(name="pcv", bufs=4, space="PSUM") as pcv:
                projt = prj.tile([128, NCH, H], F32R)
                nc.sync.dma_start(out=projt[:],
                                  in_=t["PROJT"][:].rearrange("c p o -> p c o"))
                PAD = 96
                h = [hcv.tile([128, PAD + S], F32R, tag=f"h{c}", name=f"h{c}") for c in range(NCH)]
                for c in range(NCH):
                    nc.sync.dma_start(out=h[c][:, 0:PAD], in_=t["ZPAD"][:])
                for tt in range(NTT):
                    ln_norm_tt(pstat1, x0, 0, h, tt, dst_off=PAD)

                for l, d in enumerate(STACK_DIL):
                    wl = wcv.tile([128, NCH, K, H], F32R, tag="wl")
                    nc.sync.dma_start(out=wl[:], in_=t["WST"][l])
                    hn = [hcv.tile([128, PAD + S], F32R, tag=f"h{c}", name=f"h{c}") for c in range(NCH)]
                    for c in range(NCH):
                        nc.sync.dma_start(out=hn[c][:, 0:PAD], in_=t["ZPAD"][:])
                    for tt in range(NTT):
                        for oc in range(NCH):
                            psum = pcv.tile([128, 512], F32, tag="cv")
                            i = 0
                            for k in range(K - 1, -1, -1):
                                src0 = PAD + tt * 512 - (K - 1 - k) * d
                                for ic in range(NCH):
                                    nc.tensor.matmul(
                                        psum[:], wl[:, ic, k, ts(oc, 128)],
                                        h[ic][:, src0:src0 + 512],
                                        start=(i == 0), stop=(i == 4 * NCH - 1))
                                    i += 1
                            gel = gelp.tile([128, 512], F32, tag="gel")
                            nc.scalar.activation(
                                out=gel[:], in_=psum[:], func=AF.Gelu,
                                bias=consts[:, 3 * l + oc:3 * l + oc + 1], scale=1.0)
                            nc.vector.tensor_add(
                                out=hn[oc][:, bass.ds(PAD + tt * 512, 512)],
                                in0=h[oc][:, bass.ds(PAD + tt * 512, 512)], in1=gel[:])
                    h = hn
                if dbg:
                    for c in range(NCH):
                        nc.sync.dma_start(
                            out=t["DH"][c, :, :].bitcast(F32R),
                            in_=h[c][:, PAD:PAD + S])

                # x1 = x0 + h @ proj.T + proj_b, with LN2 interleaved
                x1 = [trunk.tile([128, S], F32R, tag=f"x{c}", name=f"x{c}") for c in range(NCH)]
                n2 = [n2p.tile([128, S], BF16, tag=f"n2{c}", name=f"n2{c}") for c in range(NCH)]
                for tt in range(NTT):
                    for oc in range(NCH):
                        psum = pcv.tile([128, 512], F32, tag="cv")
                        for ic in range(NCH):
                            nc.tensor.matmul(
                                psum[:], projt[:, ic, ts(oc, 128)],
                                h[ic][:, bass.ds(PAD + tt * 512, 512)],
                                start=(ic == 0), stop=(ic == NCH - 1))
                        nc.vector.scalar_tensor_tensor(
                            out=x1[oc][:, ts(tt, 512)], in0=psum[:],
                            scalar=consts[:, 33 + oc:34 + oc],
                            in1=x0[oc][:, ts(tt, 512)],
                            op0=ALU.add, op1=ALU.add)
                    ln_norm_tt(pstat1, x1, 1, n2, tt)
            if dbg:
                dump_cm(x1, t["DX1"])

            # ---- Phase 2: multi-head dilated state branch ----
            with tc.tile_pool(name="hst", bufs=2) as hst, \
                 tc.tile_pool(name="bw", bufs=1) as bw, \
                 tc.tile_pool(name="pstat2", bufs=2, space="PSUM") as pstat2:
                st = [hst.tile([128, S], F32R, tag=f"st{c}", name=f"stt{c}") for c in range(4)]
                with tc.tile_pool(name="gwpp", bufs=1) as gwpp, \
                     tc.tile_pool(name="sgp", bufs=2) as sgp, \
                     tc.tile_pool(name="pg", bufs=2, space="PSUM") as pg:
                    gwp = gwpp.tile([128, NCH, 1024], BF16)
                    for ic in range(NCH):
                        nc.sync.dma_start(out=gwp[:, ic, :], in_=t["GWP"][ic])
                    hwa = bw.tile([128, 4, 3, K, 64], F32R)
                    nc.sync.dma_start(out=hwa[:],
                                      in_=t["HWPA"][:].rearrange("c p j k d -> p c j k d"))
                    hwb = bw.tile([128, 4, 3, K, 64], F32R)
                    nc.sync.dma_start(out=hwb[:],
                                      in_=t["HWPB"][:].rearrange("c p j k d -> p c j k d"))
                    for tt in range(NTT):
                        for vc in range(4):
                            psv = pg.tile([128, 512], F32, tag="gv")
                            pss = pg.tile([128, 512], F32, tag="gs")
                            for ic in range(NCH):
                                nc.tensor.matmul(
                                    psv[:], gwp[:, ic, ts(vc, 128)],
                                    n2[ic][:, ts(tt, 512)],
                                    start=(ic == 0), stop=(ic == NCH - 1))
                            for ic in range(NCH):
                                nc.tensor.matmul(
                                    pss[:], gwp[:, ic, 512 + 128 * vc:512 + 128 * (vc + 1)],
                                    n2[ic][:, ts(tt, 512)],
                                    start=(ic == 0), stop=(ic == NCH - 1))
                            # st = 2*glu = (tanh(s/2)+1)*v; 2x folded into
                            # head biases and mix weights host-side
                            sg = sgp.tile([128, 512], F32, tag="sg")
                            nc.scalar.activation(out=sg[:], in_=pss[:],
                                                 func=AF.Tanh, scale=0.5)
                            nc.vector.scalar_tensor_tensor(
                                out=st[vc][:, ts(tt, 512)], in0=sg[:],
                                scalar=1.0, in1=psv[:],
                                op0=ALU.add, op1=ALU.mult)

                with tc.tile_pool(name="mwp", bufs=1) as mwp, \
                     tc.tile_pool(name="ph", bufs=2, space="PSUM") as ph:
                    mixw = mwp.tile([128, 4, H], F32R)
                    nc.sync.dma_start(out=mixw[:],
                                      in_=t["MIXW"][:].rearrange("c p o -> p c o"))
                    for j in range(3):
                        stn = [hst.tile([128, S], F32R, tag=f"st{c}", name=f"stt{c}") for c in range(4)]
                        for sc in range(4):
                            dA = HEAD_DIL[2 * sc][j]
                            dB = HEAD_DIL[2 * sc + 1][j]
                            for tt in range(NTT):
                                # A head uses PE row strips 0-1 (rows 0:48),
                                # B head row strips 2-3 (rows 64:112): the
                                # interleaved matmuls run concurrently in
                                # the array, each into its own PSUM bank.
                                psA = ph.tile([128, 512], F32, tag="hda")
                                psB = ph.tile([128, 512], F32, tag="hdb")
                                mms = []
                                for k in range(K - 1, -1, -1):
                                    wnA = _win(tt, (K - 1 - k) * dA)
                                    if wnA is not None:
                                        mms.append((0, k) + wnA)
                                    wnB = _win(tt, (K - 1 - k) * dB)
                                    if wnB is not None:
                                        mms.append((1, k) + wnB)
                                nA = sum(1 for m in mms if m[0] == 0)
                                nB = len(mms) - nA
                                iA = iB = 0
                                for hb, k, src, n, dst in mms:
                                    if hb == 0:
                                        lhs = hwa[0:48, sc, j, k, :]
                                        rhs = st[sc][0:48, src:src + n]
                                        out = psA[0:64, dst:dst + n]
                                        sta, stp = iA == 0, iA == nA - 1
                                        iA += 1
                                    else:
                                        lhs = hwb[64:112, sc, j, k, :]
                                        rhs = st[sc][64:112, src:src + n]
                                        out = psB[0:64, dst:dst + n]
                                        sta, stp = iB == 0, iB == nB - 1
                                        iB += 1
                                    if n % 2 or dst % 2:
                                        lhs = lhs.bitcast(F32)
                                        rhs = rhs.bitcast(F32)
                                    nc.tensor.matmul(out, lhs, rhs,
                                                     start=sta, stop=stp)
                                bi = 18 + 3 * sc + j
                                nc.vector.scalar_tensor_tensor(
                                    out=stn[sc][0:64, ts(tt, 512)],
                                    in0=psA[0:64, :],
                                    scalar=consts[0:64, bi:bi + 1],
                                    in1=st[sc][0:64, ts(tt, 512)],
                                    op0=ALU.add, op1=ALU.add)
                                nc.vector.scalar_tensor_tensor(
                                    out=stn[sc][64:128, ts(tt, 512)],
                                    in0=psB[0:64, :],
                                    scalar=consts[64:128, bi:bi + 1],
                                    in1=st[sc][64:128, ts(tt, 512)],
                                    op0=ALU.add, op1=ALU.add)
                        st = stn
                    if dbg:
                        dump_cm(st, t["DST"])

                    # x2 = x1 + mix(state) + mix_b, with LN3 interleaved
                    # (n3 reuses the dead n2 tiles)
                    x2 = [trunk.tile([128, S], F32R, tag=f"x{c}", name=f"x{c}") for c in range(NCH)]
                    n3 = [n2p.tile([128, S], BF16, tag=f"n2{c}", name=f"n3{c}") for c in range(NCH)]
                    for tt in range(NTT):
                        for oc in range(NCH):
                            psum = ph.tile([128, 512], F32, tag="hda")
                            for sc in range(4):
                                nc.tensor.matmul(
                                    psum[:], mixw[:, sc, ts(oc, 128)],
                                    st[sc][:, ts(tt, 512)],
                                    start=(sc == 0), stop=(sc == 3))
                            nc.vector.scalar_tensor_tensor(
                                out=x2[oc][:, ts(tt, 512)], in0=psum[:],
                                scalar=consts[:, 30 + oc:31 + oc],
                                in1=x1[oc][:, ts(tt, 512)],
                                op0=ALU.add, op1=ALU.add)
                        ln_norm_tt(pstat2, x2, 2, n3, tt)
            if dbg:
                dump_cm(x2, t["DX2"])

            # ---- Phase 3: GLU FFN branch ----
            with tc.tile_pool(name="fw", bufs=1) as fw, \
                 tc.tile_pool(name="prod", bufs=1) as prodp, \
                 tc.tile_pool(name="fsgp", bufs=2) as fsgp, \
                 tc.tile_pool(name="pf", bufs=2, space="PSUM") as pf, \
                 tc.tile_pool(name="po", bufs=2, space="PSUM") as po:
                fit = fw.tile([128, NCH, 2 * INNER], BF16)
                for ic in range(NCH):
                    nc.sync.dma_start(out=fit[:, ic, :], in_=t["FIT"][ic])
                fot = fw.tile([128, NPC, H], F32R)
                nc.sync.dma_start(out=fot[:],
                                  in_=t["FOT"][:].rearrange("c p o -> p c o"))
                x3 = [trunk.tile([128, S], F32R, tag=f"x{c}", name=f"x{c}") for c in range(NCH)]
                for tt in range(NTT):
                    prods = []
                    for pc in range(NPC):
                        psv = pf.tile([128, 512], F32, tag="fv")
                        pss = pf.tile([128, 512], F32, tag="fs")
                        for ic in range(NCH):
                            nc.tensor.matmul(
                                psv[:], fit[:, ic, ts(pc, 128)],
                                n3[ic][:, ts(tt, 512)],
                                start=(ic == 0), stop=(ic == NCH - 1))
                        for ic in range(NCH):
                            nc.tensor.matmul(
                                pss[:],
                                fit[:, ic, INNER + 128 * pc:INNER + 128 * (pc + 1)],
                                n3[ic][:, ts(tt, 512)],
                                start=(ic == 0), stop=(ic == NCH - 1))
                        # 2*glu; 0.5 folded into fot host-side
                        sg = fsgp.tile([128, 512], F32, tag="fsg")
                        nc.scalar.activation(out=sg[:], in_=pss[:],
                                             func=AF.Tanh, scale=0.5)
                        pr = prodp.tile([128, 512], F32R, tag=f"pr{pc}")
                        nc.vector.scalar_tensor_tensor(
                            out=pr[:], in0=sg[:], scalar=1.0, in1=psv[:],
                            op0=ALU.add, op1=ALU.mult)
                        prods.append(pr)
                    for oc in range(NCH):
                        psum = po.tile([128, 512], F32, tag="fo")
                        for pc in range(NPC):
                            nc.tensor.matmul(
                                psum[:], fot[:, pc, ts(oc, 128)], prods[pc][:],
                                start=(pc == 0), stop=(pc == NPC - 1))
                        nc.vector.tensor_add(
                            out=x3[oc][:, ts(tt, 512)],
                            in0=x2[oc][:, ts(tt, 512)], in1=psum[:])

            # ---- Phase 4: transpose x3 back to [S, H] and store ----
            with tc.tile_pool(name="p4", bufs=4) as p4, \
                 tc.tile_pool(name="ps4", bufs=4, space="PSUM") as ps4:
                for tch in range(S // 128):
                    xo = p4.tile([128, H], F32, tag="xo")
                    for oc in range(NCH):
                        pt = ps4.tile([128, 128], F32R, tag="pt4")
                        nc.tensor.transpose(
                            pt[:], x3[oc][:, ts(tch, 128)], ident[:])
                        nc.scalar.copy(out=xo[:, ts(oc, 128)], in_=pt[:])
                    nc.sync.dma_start(out=t["OUT"][ts(tch, 128), :], in_=xo[:])

        for _rep in range(reps):
            one_pass()


def _prep_weights(inputs):
    """Host-side packing of weights into the DRAM layouts the kernel expects."""
    f = np.float32
    conv_w = np.asarray(inputs["conv_w"], f)          # [6, O, I, K]
    # lhsT[p_in, ic, k, o]: partition = input channel within chunk
    wst = np.ascontiguousarray(
        conv_w.transpose(0, 2, 3, 1).reshape(6, NCH, 128, K, H)
        .transpose(0, 2, 1, 3, 4))
    projt = np.ascontiguousarray(
        np.asarray(inputs["conv_proj_w"], f).T).reshape(NCH, 128, H)

    gate_w = np.asarray(inputs["gate_w"], f)          # [2H, H]
    gwp = np.zeros((H, 1024), f)
    for i in range(NH):
        col = 128 * (i // 2) + 64 * (i % 2)
        gwp[:, col:col + HD] = gate_w[HD * i:HD * (i + 1), :].T
        gwp[:, 512 + col:512 + col + HD] = gate_w[H + HD * i:H + HD * (i + 1), :].T
    gwp = np.ascontiguousarray(gwp.reshape(H, 1024)).reshape(NCH, 128, 1024).astype(ml_dtypes.bfloat16)

    head_w = np.asarray(inputs["head_w"], f)          # [NH, 3, HD, HD, K]
    hwpa = np.zeros((4, 128, 3, K, 64), f)
    hwpb = np.zeros((4, 128, 3, K, 64), f)
    for sc in range(4):
        # lhsT[in, j, k, out] = head_w[hi, j, out, in, k]
        hwpa[sc, 0:HD, :, :, 0:HD] = head_w[2 * sc].transpose(2, 0, 3, 1)
        hwpb[sc, 64:64 + HD, :, :, 0:HD] = head_w[2 * sc + 1].transpose(2, 0, 3, 1)
    mix_w = np.asarray(inputs["mix_w"], f)            # [H, H]
    mixw = np.zeros((4, 128, H), f)
    mt = np.ascontiguousarray(mix_w.T) * 0.5          # 2x-GLU compensation
    for sc in range(4):
        mixw[sc, 0:48] = mt[96 * sc:96 * sc + 48, :]
        mixw[sc, 64:112] = mt[96 * sc + 48:96 * (sc + 1), :]

    fit = np.ascontiguousarray(
        np.asarray(inputs["ffn_in_w"], f).T).reshape(
        NCH, 128, 2 * INNER).astype(ml_dtypes.bfloat16)
    fot = np.ascontiguousarray(
        np.asarray(inputs["ffn_out_w"], f).T * 0.5).reshape(NPC, 128, H)

    consts = np.zeros((128, NCONST), f)
    conv_b = np.asarray(inputs["conv_b"], f).reshape(6, NCH, 128)
    for l in range(6):
        for oc in range(NCH):
            consts[:, 3 * l + oc] = conv_b[l, oc]
    head_b = np.asarray(inputs["head_b"], f)          # [NH, 3, HD]
    for sc in range(4):
        for j in range(3):
            # 2x: state carries 2*glu until the mix projection
            consts[0:48, 18 + 3 * sc + j] = head_b[2 * sc, j] * 2.0
            consts[64:112, 18 + 3 * sc + j] = head_b[2 * sc + 1, j] * 2.0
    mix_b = np.asarray(inputs["mix_b"], f).reshape(NCH, 128)
    proj_b = np.asarray(inputs["conv_proj_b"], f).reshape(NCH, 128)
    for oc in range(NCH):
        consts[:, 30 + oc] = mix_b[oc]
        consts[:, 33 + oc] = proj_b[oc]

    lng = np.zeros((128, 9), f)
    lnb = np.zeros((128, 9), f)
    apply_lnga = []
    for li, (g, b) in enumerate([("ln1_g", "ln1_b"), ("ln2_g", "ln2_b"),
                                 ("ln3_g", "ln3_b")]):
        gv = np.asarray(inputs[g], f)
        bv = np.asarray(inputs[b], f)
        apply_lnga.append(not (np.all(gv == 1.0) and np.all(bv == 0.0)))
        lng[:, 3 * li:3 * li + 3] = gv.reshape(NCH, 128).T
        lnb[:, 3 * li:3 * li + 3] = bv.reshape(NCH, 128).T

    return {
        "WST": wst, "PROJT": projt, "GWP": gwp, "HWPA": hwpa, "HWPB": hwpb,
        "MIXW": mixw,
        "FIT": fit, "FOT": fot, "CONSTS": consts, "LNG": lng, "LNB": lnb,
        "ONESR": np.ones((128, 1), f), "IDENT": np.eye(128, dtype=f),
        "ZPAD": np.zeros((128, 96), f),
    }, tuple(apply_lnga)


_CACHE = {}


def _run(inputs, dbg=False, reps=1):
    x = np.asarray(inputs["x"], np.float32)           # [B, S, H]
    B = x.shape[0]
    w, apply_lnga = _prep_weights(inputs)
    key = (apply_lnga, dbg, reps)
    if key not in _CACHE:
        _CACHE[key] = _build(apply_lnga, dbg, reps)
    nc = _CACHE[key]
    in_maps = [dict(w, X=np.ascontiguousarray(x[i])) for i in range(B)]
    return run_bass_kernel_spmd(nc, in_maps, core_ids=list(range(B)))


def kernel(**inputs):
    res = _run(inputs)
    B = np.asarray(inputs["x"]).shape[0]
    return np.stack([res.results[i]["OUT"] for i in range(B)]).astype(np.float32)
